# revision 24
# baseline (speedup 1.0000x reference)
"""Trainium2 kernel for nn_DeformableConvolution1D_60636348285726.

Problem structure (hardcoded): x [4,256,4096,1], offset/mod convs 256->5 with
kernel (5,1), main conv 256->256 kernel (5,1), stride 1, height pad 2,
width pad 1 (so output width is 3).

Key mathematical simplification (exact, holds for ANY input values):
  * The width-1 input is padded to width 3. Output width positions 0 and 2 of
    the offset/modulation convs sample only zero padding, so there
    dy = offset_b[k] and mask = sigmoid(mod_b[k]) -- constants per tap.
  * Bilinear sampling x-coords are 0,1,2 for the three output width
    positions. Valid x range is [0,0]: position 0 samples the real column
    with weight 1; positions 1 and 2 sample entirely out of range -> zero
    patches -> output planes 1,2 are exactly conv_b.
  * Therefore plane 0 is an ordinary dense 1D conv along T whose effective
    taps are built on the host from offset_b / mod_b / conv_w:
        for each k: tap (k + floor(ob_k))   gets s_k*(1-frac(ob_k))*conv_w[:,:,k]
                    tap (k + floor(ob_k)+1) gets s_k*frac(ob_k)    *conv_w[:,:,k]
    with s_k = sigmoid(mod_b[k]), sampling index h - 2 + tap, zero padded.

Device kernel: dense 1D conv [B=4, C=256, T=4096] -> [4, 256, 4096] with a
Ke-tap [256,256,Ke] effective kernel, run as PSUM-accumulated 128x128x512
f32r matmuls.

v3 sharding: 8 cores = 4 batches x 2 HALVES OF C_OUT (not T halves).
Each core computes its 128 output channels over the full T=4096 using 8
PSUM banks of 512. Per-core weights are then only [256, Ke*128] -- the
piece of data that gates the start of the matmul stream (weights for
cin-tile 0 plus the first x chunk) shrinks to ~0.66 MB, so the stream
starts ~4 us earlier than with T sharding. x chunks stream in
chunk-granular pieces, each with its own semaphore, alternating across
the two HWDGE rings in exact need-order, so the PE never waits on data
that is still queued behind not-yet-needed bytes.

Other tricks (vs the naive version):
  * walrus --enable-ldw-opt=true (patched in at compile time).
  * Outputs evicted from PSUM as bf16 (host converts back): halves output
    DMA bytes. The final PSUM bank's eviction + DMA are split in half
    across the sync and scalar queues to shorten the tail.
  * fp32 junk-matmul warmup ramps the PE clock to 2.4 GHz while the first
    input DMA is in flight.
  * Block(no_gpsimd_drain=True): skips the GPSIMD dge drain at exit.
"""

import os
import numpy as np

# Problem constants (hardcoded per the task contract).
B, CIN, COUT, T, W = 4, 256, 256, 4096, 1
K, PAD = 5, 2
NCORES = 8
CO = 128             # per-core output channels (C_out half)
NFREE = 512          # matmul moving free size / PSUM bank
P = 128              # partition dim
NCH = T // NFREE     # 8 chunks of 512

WARM = int(os.environ.get("DEFORM_WARM", "11"))       # warmup matmul count
LDW_OPT = os.environ.get("DEFORM_LDW", "1") == "1"    # walrus ldw dedup
OUT_BF16 = os.environ.get("DEFORM_OUT", "bf16") == "bf16"
NOBAR = os.environ.get("DEFORM_BAR", "0") != "1"      # skip init/exit barriers
ENDWAIT = os.environ.get("DEFORM_ENDWAIT", "0") == "1"  # explicit final waits

_PROGRAM_CACHE = {}


def _patch_ldw_opt():
    """Turn on walrus's redundant-LDWEIGHTS elimination (concourse pins it
    off). Safe here: weights live in SBUF columns written once by the input
    DMA before first use, and all PE waits are standalone instructions."""
    import concourse.bass_utils as _bu
    if getattr(_bu, "_deform_ldw_patch", False):
        return
    _orig = _bu.run_command

    def run_command_ldw(cmd, *a, **kw):
        cmd = [
            ("--enable-ldw-opt=true" if c == "--enable-ldw-opt=false" else c)
            for c in cmd
        ]
        return _orig(cmd, *a, **kw)

    _bu.run_command = run_command_ldw
    _bu._deform_ldw_patch = True


def _build_program(Ke: int):
    """Build the per-core Bass program (identical on all 8 cores).

    Raw bass (no Tile): every cross-engine dependency is an explicit
    standalone wait instruction.

    Per-core dataflow:
      sync/scalar: input pieces in need-order, alternating between the two
              HWDGE rings, all issued back-to-back (each ring drains its
              entries in order); then per closed PSUM bank: SBUF->DRAM out.
      tensor: fp32 junk warmup (clock ramp) while the first input piece is
              in flight, then per (ci, chunk): wait that piece's sem ->
              Ke accumulating matmuls.
      vector: evict each PSUM bank to SBUF (casting to bf16) as its
              accumulation group closes; last bank in 2 halves.
    """
    import concourse.bass as bass
    from concourse import mybir

    f32 = mybir.dt.float32
    bf16 = mybir.dt.bfloat16
    mmdt = mybir.dt.float32r
    outdt = bf16 if OUT_BF16 else f32

    XL = T + Ke - 1           # x columns in the slab
    WL = Ke * CO              # weight columns (stored FIRST in the slab)
    SL = WL + XL              # slab columns
    HB = NFREE // 2           # half-bank columns (last-bank split)

    # The Bass init/exit all-engine barriers cost ~4 us of serial semaphore
    # chains. They only order the const-AP memsets (which none of our ops
    # read) and align engine starts; all of our cross-engine dependencies
    # flow through explicit semaphores, which NRT zeroes per execution, and
    # each engine's exit InstDrain still fences its in-flight DMAs. So the
    # barriers are pure overhead here -- drop them and the first input DMA
    # issues ~5 us earlier.
    orig_barrier = bass.Bass.all_engine_barrier
    if NOBAR:
        bass.Bass.all_engine_barrier = lambda self, *, sem_only=False: None
    try:
        nc = bass.Bass("TRN2", target_bir_lowering=False, debug=False)
    finally:
        bass.Bass.all_engine_barrier = orig_barrier
    if NOBAR:
        nc.all_engine_barrier = lambda *, sem_only=False: None

    xw = nc.dram_tensor("xw", [CIN, SL], mmdt, kind="ExternalInput").ap()
    out = nc.dram_tensor("out", [CO, T], outdt, kind="ExternalOutput").ap()

    CI_T = CIN // P           # 2 cin partition tiles

    with (
        nc.sbuf_tensor([P, SL], mmdt) as xw0,
        nc.sbuf_tensor([P, SL], mmdt) as xw1,
        nc.sbuf_tensor([P, T], outdt) as ot,
        nc.sbuf_tensor([P, P], f32) as warm,
        nc.psum_tensor([P, NCH, NFREE], f32) as pt,
        nc.semaphore("pe_sem") as pe_sem,
        nc.semaphore("dve_sem") as dve_sem,
        nc.semaphore("gve_sem") as gve_sem,
        nc.semaphore("out_sem") as out_sem,
        nc.semaphore("warm_sem") as warm_sem,
        nc.Block(no_gpsimd_drain=True) as block,
    ):
        xw_sb = [xw0, xw1]

        # ---- input piece schedule, in exact need-order ----
        # piece = (sbuf, ci, c0, c1); first piece (weights ci0 + chunk 0)
        # is split in half across both queues, the rest alternate whole.
        head = WL + NFREE + Ke - 1          # cols needed by (ci, chunk 0)
        x_pieces = []                        # [(ci, c0, c1, chunk)]
        for ci in range(CI_T):
            if ci > 0:
                x_pieces.append((ci, 0, head, 0))
            for ch in range(1, NCH):
                c0 = WL + ch * NFREE + Ke - 1
                c1 = min(SL, WL + (ch + 1) * NFREE + Ke - 1)
                x_pieces.append((ci, c0, c1, ch))

        n_sems = 1 + len(x_pieces)
        stage_sems = [
            nc.ctx.enter_context(nc.semaphore(f"sp{i}")) for i in range(n_sems)
        ]
        # need_sem[(ci, ch)] -> (sem, threshold)
        need_sem = {(0, 0): (stage_sems[0], 32)}
        for i, (ci, c0, c1, ch) in enumerate(x_pieces):
            need_sem[(ci, ch)] = (stage_sems[1 + i], 16)
        for ci in range(CI_T):
            for ch in range(NCH):
                if (ci, ch) not in need_sem:
                    need_sem[(ci, ch)] = need_sem[(ci, ch - 1)]

        sync_pieces = [(xw0, 0, 0, head // 2, stage_sems[0])]
        scal_pieces = [(xw0, 0, head // 2, head, stage_sems[0])]
        for i, (ci, c0, c1, ch) in enumerate(x_pieces):
            dst = sync_pieces if i % 2 == 0 else scal_pieces
            dst.append((xw_sb[ci], ci, c0, c1, stage_sems[1 + i]))

        # Bank closure order (stop matmuls fire in chunk order during ci1).
        closure = list(range(NCH))

        @block.sync
        def _(sync):
            for sb, ci, c0, c1, sem in sync_pieces:
                sync.dma_start(
                    out=sb[:, c0:c1],
                    in_=xw[ci * P:(ci + 1) * P, c0:c1],
                ).then_inc(sem, 16)
            for k in closure[:-1]:
                sync.wait_ge(dve_sem, k + 1)
                sync.dma_start(
                    out=out[:, k * NFREE:(k + 1) * NFREE],
                    in_=ot[:, k * NFREE:(k + 1) * NFREE],
                ).then_inc(out_sem, 16)
            # Last bank split in half across both queues so the final
            # transfers ride in parallel. No final out_sem wait: the exit
            # InstDrain on each queue fences its in-flight DMAs (verified by
            # the run-to-run exact-compare in the harness).
            lch = closure[-1]
            sync.wait_ge(dve_sem, NCH)
            sync.dma_start(
                out=out[:, lch * NFREE:lch * NFREE + HB],
                in_=ot[:, lch * NFREE:lch * NFREE + HB],
            ).then_inc(out_sem, 16)
            if ENDWAIT:
                sync.wait_ge(out_sem, (NCH + 1) * 16)

        @block.scalar
        def _(scalar):
            for sb, ci, c0, c1, sem in scal_pieces:
                scalar.dma_start(
                    out=sb[:, c0:c1],
                    in_=xw[ci * P:(ci + 1) * P, c0:c1],
                ).then_inc(sem, 16)
            lch = closure[-1]
            scalar.wait_ge(gve_sem, 1)
            scalar.dma_start(
                out=out[:, lch * NFREE + HB:(lch + 1) * NFREE],
                in_=ot[:, lch * NFREE + HB:(lch + 1) * NFREE],
            ).then_inc(out_sem, 16)
            if ENDWAIT:
                scalar.wait_ge(out_sem, (NCH + 1) * 16)

        @block.tensor
        def _(tensor):
            # Clock-ramp warmup: fp32 junk matmuls (4 cyc/row) while the
            # first input piece is in flight.
            tensor.wait_ge(warm_sem, 1)
            for _ in range(WARM):
                nc.tensor.matmul(
                    pt[:, 0, 0:P],
                    lhsT=warm[:, :],
                    rhs=warm[:, :],
                    start=True,
                    stop=True,
                )
            # Chunk-group passes, j-outer with the group interleaved:
            # consecutive matmuls alternate PSUM banks (no same-bank
            # back-to-back turnaround) and share lhsT (walrus ldw-opt
            # drops the repeat loads). ci0's passes are sized to the DMA
            # piece arrival cadence (chunk 0 solo so the stream starts as
            # soon as the first weights+chunk0 piece lands); by the time
            # ci1 runs, all its pieces have landed, so wider passes
            # minimize wait-boundary pipeline bubbles.
            # ci0's passes are sized to the DMA piece arrival cadence
            # (chunk 0 solo so the stream starts as soon as the first
            # weights+chunk0 piece lands). ci1's pieces have all landed by
            # the time it runs, so it pairs every pass: solo passes run 6
            # consecutive same-bank matmuls, which measure ~20 ns/matmul
            # slower than alternating banks.
            ci_passes = [
                [(0,), (1, 2), (3, 4), (5, 6), (7,)],
                [(0, 1), (2, 3), (4, 5), (6, 7)],
            ]
            for ci in range(CI_T):
                src = xw_sb[ci]
                for chs in ci_passes[ci]:
                    sem, thr = need_sem[(ci, chs[-1])]
                    tensor.wait_ge(sem, thr)
                    for j in range(Ke):
                        start = (ci == 0 and j == 0)
                        stop = (ci == CI_T - 1 and j == Ke - 1)
                        for ch in chs:
                            mm = nc.tensor.matmul(
                                pt[:, ch, :],
                                lhsT=src[:, j * CO: j * CO + P],
                                rhs=src[:, WL + ch * NFREE + j:
                                        WL + ch * NFREE + j + NFREE],
                                start=start,
                                stop=stop,
                            )
                            if stop:
                                mm.then_inc(pe_sem, 1)

        @block.vector
        def _(vector):
            nc.vector.memset(warm[:, :], 0.0).then_inc(warm_sem, 1)
            # Evict each PSUM bank to SBUF (casting to outdt) as soon as its
            # accumulation group closes. Last bank in two halves so its
            # output DMA can start earlier and split across two queues.
            for k in closure[:-1]:
                vector.wait_ge(pe_sem, k + 1)
                nc.vector.tensor_copy(
                    ot[:, k * NFREE:(k + 1) * NFREE],
                    pt[:, k, :],
                ).then_inc(dve_sem, 1)
            lch = closure[-1]
            vector.wait_ge(pe_sem, NCH)
            nc.vector.tensor_copy(
                ot[:, lch * NFREE:lch * NFREE + HB],
                pt[:, lch, 0:HB],
            ).then_inc(dve_sem, 1)
            nc.vector.tensor_copy(
                ot[:, lch * NFREE + HB:(lch + 1) * NFREE],
                pt[:, lch, HB:NFREE],
            ).then_inc(gve_sem, 1)

    return nc


def _effective_taps(offset_b, mod_b, conv_w3):
    """Collapse offsets/modulation/conv_w into an effective conv kernel.

    Returns (E [COUT, CIN, Ke] f32, tmin) where plane-0 output is
    out0[b,o,h] = sum_{j,c} E[o,c,j] * xzero[b,c,h-PAD+tmin+j] + conv_b[o].
    """
    ob = offset_b.astype(np.float64)
    f = np.floor(ob).astype(np.int64)
    w1 = ob - f
    w0 = 1.0 - w1
    s = 1.0 / (1.0 + np.exp(-mod_b.astype(np.float64)))

    tmin = int(min(k + f[k] for k in range(K)))
    tmax = int(max(k + f[k] + 1 for k in range(K)))
    Ke = tmax - tmin + 1
    E = np.zeros((COUT, CIN, Ke), np.float64)
    cw = conv_w3.astype(np.float64)
    for k in range(K):
        E[:, :, k + f[k] - tmin] += cw[:, :, k] * (s[k] * w0[k])
        E[:, :, k + f[k] + 1 - tmin] += cw[:, :, k] * (s[k] * w1[k])
    return E.astype(np.float32), tmin


def _run(inputs, trace=False, tmpdir=None):
    if LDW_OPT:
        _patch_ldw_opt()
    from concourse.bass_utils import run_bass_kernel_spmd

    x = np.asarray(inputs["x"], np.float32)
    offset_b = np.asarray(inputs["offset_b"], np.float32)
    mod_b = np.asarray(inputs["mod_b"], np.float32)
    conv_w = np.asarray(inputs["conv_w"], np.float32)
    conv_b = np.asarray(inputs["conv_b"], np.float32)
    assert x.shape == (B, CIN, T, W), x.shape

    x3 = np.ascontiguousarray(x[:, :, :, 0])            # [B,C,T]
    conv_w3 = np.ascontiguousarray(conv_w[:, :, :, 0])  # [O,C,K]

    E, tmin = _effective_taps(offset_b, mod_b, conv_w3)
    Ke = E.shape[2]

    # Zero-padded x so that per-core slabs are uniform:
    # xp[:, :, i] = x[:, :, i - L] (zero outside), L = PAD - tmin.
    L = PAD - tmin
    Tp = T + Ke - 1
    xp = np.zeros((B, CIN, Tp), np.float32)
    lo, hi = max(0, L), min(Tp, L + T)
    if lo < hi:
        xp[:, :, lo:hi] = x3[:, :, lo - L:hi - L]

    key = (Ke, WARM, LDW_OPT, OUT_BF16, NOBAR, ENDWAIT)
    if key not in _PROGRAM_CACHE:
        _PROGRAM_CACHE[key] = _build_program(Ke)
    nc = _PROGRAM_CACHE[key]

    XL = T + Ke - 1
    WL = Ke * CO
    in_maps = []
    for core in range(NCORES):
        b, coh = core // 2, core % 2
        # Weights in lhsT layout for this C_out half:
        # wt[ci, j*CO + co] = E[coh*CO + co, ci, j].
        wt = np.ascontiguousarray(
            E[coh * CO:(coh + 1) * CO].transpose(1, 2, 0).reshape(CIN, Ke * CO))
        xwm = np.empty((CIN, WL + XL), np.float32)
        xwm[:, :WL] = wt
        xwm[:, WL:] = xp[b]
        in_maps.append({"xw": xwm})

    res = run_bass_kernel_spmd(
        nc, in_maps, core_ids=list(range(NCORES)),
        trace=trace, tmpdir=tmpdir,
    )

    out = np.empty((B, COUT, T, 3), np.float32)
    out[:, :, :, 1] = conv_b[None, :, None]
    out[:, :, :, 2] = conv_b[None, :, None]
    for core in range(NCORES):
        b, coh = core // 2, core % 2
        out[b, coh * CO:(coh + 1) * CO, :, 0] = \
            np.asarray(res.results[core]["out"], dtype=np.float32)
    out[:, :, :, 0] += conv_b[None, :, None]
    return out, res


def kernel(**inputs):
    out, _ = _run(inputs, trace=False)
    return out
